# revision 67
# baseline (speedup 1.0000x reference)
"""Distributed Trainium2 Bass kernel for an attention block (fused, v4).

Reference math (B=2, S=2048, H=2048, NH=16, HD=128):
  qkv = x @ Wqkv.T -> split q,k,v per head -> RoPE(q,k via frequency_cis 2x2)
  scores = (q @ k.T) * 1/sqrt(HD) + causal mask -> softmax -> @ v -> @ Wout.T

Sharding (8 cores): core c handles batch b=c//4 and heads 4*(c%4)..4*(c%4)+3.

v4 changes over v3 (476us):
  - l (softmax denominator) no longer costs a 128x1x512 PE matmul per key
    chunk (~50us of PE): et chunks are summed element-wise on the DVE
    (partition-sum commutes with the chunk-sum), then ONE ones[128,128]
    stationary matmul per (head, slab) yields l already broadcast across
    partitions; 1/l via reciprocal_approx_fast; the normalization multiply
    reads pv straight from PSUM (pvs copies, gpsimd partition_broadcasts,
    and the scalar Ln/Exp chain all deleted).
  - AllGathers are per (slab, head) [128 rows each] and fire as soon as
    that head's at-DMA lands, so the final exposed AG is 1/4 the bytes.
  - out_proj(0)/(1)/(2) are interleaved into attention(2)/proj(3)/
    attention(3) as mq-major chains, so the post-attention tail is only
    AG(3,h3) + the head-major out_proj(3).
  - prologue DMAs strictly in first-consumption order (wqk0/1, all of
    x-slab0, rope, then the rest) to cut PE starvation at start.
"""

import numpy as np
import ml_dtypes
from contextlib import ExitStack

B, S, H, NH, HD = 2, 2048, 2048, 16, 128
NHL = 4          # heads per core
NCORES = 8
SCALE = 1.0 / np.sqrt(HD)
BF16 = ml_dtypes.bfloat16
NEG = -1e9

_cache = {}


def _build():
    import concourse.bass as bass
    import concourse.tile as tile
    from concourse import bacc, mybir
    dt = mybir.dt
    nc = bacc.Bacc("TRN2", target_bir_lowering=False, debug=False,
                   num_devices=NCORES)

    P = 128
    KO = H // P           # 16 contraction chunks for the projections
    NQT = S // 512        # 4 q tiles of 512

    xT = nc.dram_tensor("xT", [H, S], dt.bfloat16, kind="ExternalInput").ap()
    # block-major: wqkT[i] = columns [128i, 128i+128) of Wqk.T, contiguous
    # so the per-matmul-group prologue loads are single fast DMAs
    wqkT = nc.dram_tensor("wqkT", [2 * NHL, H, HD], dt.bfloat16,
                          kind="ExternalInput").ap()
    wvT = nc.dram_tensor("wvT", [H, NHL * HD], dt.bfloat16,
                         kind="ExternalInput").ap()
    rope = nc.dram_tensor("rope", [2, HD, S], dt.bfloat16,
                          kind="ExternalInput").ap()
    mtri = nc.dram_tensor("mtri", [P, P], dt.float32,
                          kind="ExternalInput").ap()
    woutT = nc.dram_tensor("woutT", [H, 512], dt.bfloat16,
                           kind="ExternalInput").ap()
    out_ext = nc.dram_tensor("out", [S, 512], dt.float32,
                             kind="ExternalOutput").ap()

    # internal DRAM for the AllGathers. One AG per slab for slabs 0-2
    # (fully hidden, and each AG carries ~5-7us of serialized CC latency so
    # fewer is better); slab 3 splits A=heads 0-2 / B=head 3 so the tail
    # exposes only the small B gather.
    atl = nc.dram_tensor("attnT_loc", [NQT, NHL, P, 512], dt.bfloat16)
    ats = nc.dram_tensor("attnT_sh", [NQT, 4 * NHL, P, 512], dt.bfloat16)

    with tile.TileContext(nc) as tc, ExitStack() as ctx:
        per = ctx.enter_context(tc.tile_pool(name="per", bufs=1))
        sb = ctx.enter_context(tc.tile_pool(name="sb", bufs=1))
        ps = ctx.enter_context(tc.tile_pool(name="ps", bufs=1, space="PSUM"))

        # persistent: roped q/k in [HD, h, S]; v natural [s%P, s//P, h*HD+d]
        qsb = per.tile([P, NHL, S], dt.bfloat16, tag="qsb")
        ksb = per.tile([P, NHL, S], dt.bfloat16, tag="ksb")
        vsb = per.tile([P, KO, NHL * HD], dt.bfloat16, tag="vsb")
        ones = per.tile([P, P], dt.bfloat16, tag="ones")
        nc.vector.memset(ones[:], 1.0)
        # warm the PE's HAM clock gate with dummy matmuls while the first
        # weight/x DMAs are in flight: activity from ~t=0 flips the 4/8
        # throttle ~2us before the first real matmul would on its own
        warm = ps.tile([P, 512], dt.float32, tag="psc", bufs=4, name="warm")
        for _ in range(24):
            nc.tensor.matmul(warm[:, 0:P], ones[:], ones[:],
                             start=True, stop=True)

        # ---- prologue loads in strict first-consumption order: the first
        # matmul group needs wqk block 0 + x chunks (streamed), then rope
        # for its rope stage, then the remaining weight blocks ----
        xTr = xT.rearrange("(ko p) s -> p ko s", p=P)
        wqkr = wqkT.rearrange("b (ko p) m -> b p ko m", p=P)
        wqk = per.tile([P, 2 * NHL, KO, HD], dt.bfloat16, tag="wqk")
        xn0 = sb.tile([P, KO, 512], dt.bfloat16, tag="xn", bufs=2)
        rsb0 = sb.tile([P, 2, 512], dt.bfloat16, tag="rsb", bufs=2)
        wv = per.tile([P, KO, NHL * HD], dt.bfloat16, tag="wv")
        msb = per.tile([P, P], dt.float32, tag="msb")
        nc.sync.dma_start(msb[:], mtri)   # tiny; warms the DMA path
        # micro-stage the first weight block: the first matmul needs only
        # kc 0-1 (64KB), not the whole 0.5MB block
        nc.sync.dma_start(wqk[:, 0, 0:2], wqkr[0][:, 0:2])
        nc.sync.dma_start(wqk[:, 0, 2:], wqkr[0][:, 2:])
        nc.sync.dma_start(wqk[:, 1], wqkr[1])
        for i, ck in enumerate([(0, 2), (2, 4), (4, 8), (8, 12), (12, 16)]):
            nc.sync.dma_start(xn0[:, slice(*ck), :], xTr[:, slice(*ck), 0:512])
            if i == 1:   # rope is consumed right after the first qk group
                nc.sync.dma_start(
                    rsb0[:], rope.rearrange("r p s -> p r s")[:, :, 0:512])
        for i in range(2, 8):
            nc.sync.dma_start(wqk[:, i], wqkr[i])
        nc.sync.dma_start(wv[:], wvT.rearrange("(ko p) m -> p ko m", p=P))
        wo = per.tile([P, KO, 512], dt.bfloat16, tag="wo")
        nc.sync.dma_start(wo[:], woutT.rearrange("(ko p) n -> p ko n", p=P))

        def proj(n, xn, rsb, interleave=None):
            ns = slice(n * 512, (n + 1) * 512)
            for h in range(NHL):
                for t in range(2):   # q, k with RoPE
                    # qk groups 1/3/5 borrow the attention-only ppv banks
                    # (idle during proj) to relieve chain-start stalls on
                    # the rope-gated pmm rotation; groups 6/7 stay on pmm
                    # so ppv's last proj use drains ~17us before attention
                    # needs it (no jitter-sensitive boundary coupling)
                    g = h * 2 + t
                    tag = "ppv" if g in (1, 3, 5) else "pmm"
                    pp = ps.tile([P, 512], dt.float32, tag=tag, bufs=2,
                                 name="pp")
                    for kc in range(KO):
                        nc.tensor.matmul(
                            pp[:], wqk[:, h * 2 + t, kc, :], xn[:, kc, :],
                            start=(kc == 0), stop=(kc == KO - 1))
                    # rope input holds [A, swap(B)]; u = q*swap(B), then
                    # DMA-swap u's partition halves so t2 = swap(q)*B,
                    # and dst = q*A + t2.
                    dst = qsb if t == 0 else ksb
                    t1 = sb.tile([P, 512], dt.bfloat16, tag="t1", bufs=2)
                    u = sb.tile([P, 512], dt.bfloat16, tag="u", bufs=2)
                    t2 = sb.tile([P, 512], dt.bfloat16, tag="t2", bufs=2)
                    nc.vector.tensor_tensor(t1[:], pp[:], rsb[:, 0, :],
                                            mybir.AluOpType.mult)
                    nc.vector.tensor_tensor(u[:], pp[:], rsb[:, 1, :],
                                            mybir.AluOpType.mult)
                    # swaps issued from Scalar (idle during proj): gpsimd
                    # blocks on collectives, and sync's at/out-DMAs would
                    # head-of-line-block these behind the finish chain
                    nc.scalar.dma_start(t2[:64], u[64:, :])
                    nc.scalar.dma_start(t2[64:], u[:64, :])
                    nc.vector.tensor_tensor(dst[:, h, ns], t1[:], t2[:],
                                            mybir.AluOpType.add)
                if interleave is not None and h % 2 == 1:
                    # slot an out_proj mq-chain pair between qk head groups
                    interleave(2 * (h // 2))
                    interleave(2 * (h // 2) + 1)
            # v in natural layout: stationary = x s-cols, moving = Wv.
            # evacuation on Scalar (idle during proj) keeps the DVE FIFO
            # clear for the rope chain.
            for j in range(4):
                # slab 0 borrows the idle attention psum so the v matmuls
                # don't contend with the rope-gated pmm banks at startup
                tag, bufs = ("psc", 4) if n == 0 else ("pmm", 2)
                pv = ps.tile([P, 512], dt.float32, tag=tag, bufs=bufs,
                             name="pv")
                for kc in range(KO):
                    nc.tensor.matmul(
                        pv[:], xn[:, kc, j * P:(j + 1) * P], wv[:, kc, :],
                        start=(kc == 0), stop=(kc == KO - 1))
                nc.scalar.copy(vsb[:, n * 4 + j, :], pv[:])

        asb_tiles = {}   # (qt, h) -> sbuf tile [P, 4, 512]; layout [p, g, s]
        deferred_pf = []

        def gather(qt, h):
            """AllGather one (slab, head) [128 rows] + prefetch. Per-head
            gathers keep each CC mesh step's latency (~10-15us serialized on
            the CC core) small and early; the last slab exposes only h3's.

            For slab 3 heads 0-1 the COLLECTIVE fires immediately (the AG
            pipeline start is the tail's binding constraint) but the
            prefetch DMA emission is deferred until head 2's gather, after
            the last out_proj(1) chain has been traced: the prefetch
            overwrites slab 1's buffer slot, and tile's WAR tracking only
            orders it after already-emitted readers."""
            nc.gpsimd.collective_compute(
                "AllGather",
                mybir.AluOpType.bypass,
                replica_groups=[[0, 1, 2, 3], [4, 5, 6, 7]],
                ins=[atl.ap()[qt, h].opt()],
                outs=[ats.ap()[qt, 4 * h:4 * h + 4].opt()],
            )
            # prefetch on gpsimd: the load waits on the AG semaphore, and a
            # sync-queue wait would head-of-line-block the rope swaps and
            # x loads queued behind it. The very last gather's prefetch
            # rides the scalar queue instead (idle after the final EXP) -
            # it is on the critical tail path and skips the gpsimd wake.
            asb = sb.tile([P, 4, 512], dt.bfloat16, tag=f"asb{h}", bufs=2,
                          name="asbh")
            src = ats.ap()[qt, 4 * h:4 * h + 4].rearrange("k p s -> p k s")
            if qt == NQT - 1 and h < 2:
                deferred_pf.append((asb, src))
            else:
                if qt == NQT - 1 and h == 2:
                    for a, s in deferred_pf:
                        nc.gpsimd.dma_start(a[:], s)
                    deferred_pf.clear()
                eng = (nc.scalar if (qt == NQT - 1 and h == NHL - 1)
                       else nc.gpsimd)
                eng.dma_start(asb[:], src)
            asb_tiles[(qt, h)] = asb

        def attention(qt, interleave=None, pre_gather=None):
            for h in range(NHL):
                nkc = 4 * qt + 4
                ppv = ps.tile([P, 512], dt.float32, tag="ppv", bufs=2,
                              name="pvacc")
                etacc = sb.tile([P, 512], dt.bfloat16, tag="etacc", bufs=2)
                emitted = []

                def flush_one():
                    kc, et, q0 = emitted.pop(0)
                    nc.tensor.matmul(ppv[:, q0:],
                                     vsb[:, kc, h * P:(h + 1) * P],
                                     et[:, q0:], start=(kc == 0),
                                     stop=(kc == nkc - 1))

                for kc in range(nkc):
                    # columns < q0 of this chunk are fully causally masked:
                    # restrict every op to the active [q0:] range (the
                    # skipped region contributes exact zeros to l and pv)
                    j = kc - 4 * qt
                    q0 = j * P if j > 0 else 0
                    sc = ps.tile([P, 512], dt.float32, tag="psc", bufs=4,
                                 name="sc")
                    nc.tensor.matmul(sc[:, q0:],
                                     ksb[:, h, kc * P:(kc + 1) * P],
                                     qsb[:, h, qt * 512 + q0:
                                         (qt + 1) * 512],
                                     start=True, stop=True)
                    et = sb.tile([P, 512], dt.bfloat16, tag="et", bufs=5)
                    if j >= 0:   # straddles the causal diagonal
                        nc.vector.tensor_tensor(
                            sc[:, j * P:(j + 1) * P],
                            sc[:, j * P:(j + 1) * P],
                            msb[:], mybir.AluOpType.add)
                    nc.scalar.activation(
                        et[:, q0:], sc[:, q0:],
                        mybir.ActivationFunctionType.Exp)
                    # DVE partial-sum for l: summing chunks element-wise
                    # commutes with the later partition sum
                    if kc == 0:
                        nc.vector.tensor_copy(etacc[:], et[:])
                    else:
                        nc.vector.tensor_tensor(etacc[:, q0:],
                                                etacc[:, q0:], et[:, q0:],
                                                mybir.AluOpType.add)
                    emitted.append((kc, et, q0))
                    while len(emitted) > 3:
                        flush_one()
                while emitted:
                    flush_one()

                # l broadcast across partitions via ones[128,128] stationary;
                # its psum comes from the psc rotation (freed bank -> bufs=4)
                lps = ps.tile([P, 512], dt.float32, tag="psc", bufs=4,
                              name="lacc")
                nc.tensor.matmul(lps[:], ones[:], etacc[:],
                                 start=True, stop=True)
                rl = sb.tile([P, 512], dt.float32, tag="rl", bufs=2)
                nc.vector.reciprocal_approx_fast(rl[:], lps[:])
                at = sb.tile([P, 512], dt.bfloat16, tag="at", bufs=2)
                nc.vector.tensor_tensor(at[:], ppv[:], rl[:],
                                        mybir.AluOpType.mult)
                nc.sync.dma_start(atl.ap()[qt, h], at[:])
                # pre_gather thunks run BEFORE this head's gather is
                # emitted: their asb reads must be traced before the
                # slab-3 prefetches overwrite the shared buffer slots
                # (tile's WAR tracking only sees already-emitted readers)
                for fn in (pre_gather or {}).get(h, []):
                    fn()
                gather(qt, h)
                for fn in (interleave or {}).get(h, []):
                    fn()

        def mk_outproj_chain(qt):
            """Returns chain(mq): one mq's full 16-MM accumulation +
            evacuation for out_proj(qt), usable as an interleave slot."""
            def chain(mq):
                po = ps.tile([P, 512], dt.float32, tag="pmm", bufs=2,
                             name="po")
                i = 0
                for h in range(NHL):
                    a = asb_tiles[(qt, h)]
                    for g in range(4):
                        nc.tensor.matmul(
                            po[:], a[:, g, mq * P:(mq + 1) * P],
                            wo[:, g * 4 + h, :],
                            start=(i == 0), stop=(i == 15))
                        i += 1
                # evacuation on DVE: a scalar-queue copy here would wait on
                # this chain's matmuls while EXPs of the surrounding
                # attention queue behind it (strict FIFO) - that stall
                # cascades into late at-DMAs and late AllGathers
                ev = sb.tile([P, 512], dt.float32, tag="ev", bufs=2)
                nc.vector.tensor_copy(ev[:], po[:])
                nc.sync.dma_start(
                    out_ext[(qt * 4 + mq) * P:(qt * 4 + mq + 1) * P, :],
                    ev[:])
            return chain

        def out_proj_final(qt):
            # all 4 accumulators live; consume the B part (head 3, early
            # gather) first, then the A heads as the big gather lands
            pos = []
            for mq in range(4):
                tag, bufs = (("pmm", 2) if mq < 2 else ("psc", 4))
                pos.append(ps.tile([P, 512], dt.float32, tag=tag,
                                   bufs=bufs, name="pof"))
            for h in range(NHL):
                a = asb_tiles[(qt, h)]
                for mq in range(4):
                    for g in range(4):
                        nc.tensor.matmul(
                            pos[mq][:], a[:, g, mq * P:(mq + 1) * P],
                            wo[:, g * 4 + h, :],
                            start=(h == 0 and g == 0),
                            stop=(h == NHL - 1 and g == 3))
                    if h == NHL - 1:
                        # evacuate each mq as soon as its chain stops,
                        # alternating scalar/vector so two copy+DMA
                        # chains drain in parallel at the very end
                        ev = sb.tile([P, 512], dt.float32, tag="ev",
                                     bufs=2)
                        orow = out_ext[(qt * 4 + mq) * P:
                                       (qt * 4 + mq + 1) * P, :]
                        if mq % 2 == 0:
                            nc.scalar.copy(ev[:], pos[mq][:])
                            nc.scalar.dma_start(orow, ev[:])
                        else:
                            nc.vector.tensor_copy(ev[:], pos[mq][:])
                            nc.sync.dma_start(orow, ev[:])

        from functools import partial
        rr = rope.rearrange("r p s -> p r s")
        xn_t, rsb_t = xn0, rsb0
        for n in range(NQT):
            # issue next slab's x/rope loads first: with xn double-buffered
            # they overlap all of proj(n)+attention(n) instead of starting
            # after proj(n) finished reading the shared buffer
            if n + 1 < NQT:
                ns2 = slice((n + 1) * 512, (n + 2) * 512)
                xn_nx = sb.tile([P, KO, 512], dt.bfloat16, tag="xn",
                                bufs=2, name="xn_n")
                nc.sync.dma_start(xn_nx[:], xTr[:, :, ns2])
                rsb_nx = sb.tile([P, 2, 512], dt.bfloat16, tag="rsb",
                                 bufs=2, name="rsb_n")
                nc.sync.dma_start(rsb_nx[:], rr[:, :, ns2])
            proj(n, xn_t, rsb_t)
            il, pg = None, None
            if n == 2:
                c0 = mk_outproj_chain(0)
                il = {h: [partial(c0, h)] for h in range(NHL)}
            elif n == 3:
                # out_proj(1) chains spread across heads 0-2 (emitted as
                # pre-gather thunks, before each head's own pf): the AG
                # inputs for h1/h2 then land earlier relative to the
                # slab's end and the serial ~14us/gather CC drain delivers
                # the last gather ~T+18 instead of ~T+27
                c1 = mk_outproj_chain(1)
                pg = {0: [partial(c1, 0), partial(c1, 1)],
                      1: [partial(c1, 2)], 2: [partial(c1, 3)]}
            attention(n, interleave=il, pre_gather=pg)
            if n + 1 < NQT:
                xn_t, rsb_t = xn_nx, rsb_nx
        # slab 3's per-head AllGathers drain serially on the CC core at
        # ~12-14us each from the first at-DMA; attention(3) runs pure (so
        # the gather inputs land as early as possible) and out_proj(2) +
        # OPF's h0-h2 groups fill until h3's gather lands
        c2 = mk_outproj_chain(2)
        for mq in range(4):
            c2(mq)
        out_proj_final(NQT - 1)

    nc.compile()
    return nc


def _host_prep(x, attention_mask, frequency_cis, Wqkv, Wout):
    """Build the 8 per-core input maps (numpy only)."""
    x = np.asarray(x, dtype=np.float32)
    fc = np.asarray(frequency_cis, dtype=np.float32)
    Wqkv = np.asarray(Wqkv, dtype=np.float32)
    Wout = np.asarray(Wout, dtype=np.float32)

    # rotate-half permutation of the head dim: new row p<64 <- old 2p,
    # p>=64 <- old 2(p-64)+1
    perm = np.concatenate([np.arange(0, HD, 2), np.arange(1, HD, 2)])
    # rope coefficients in permuted layout: [A;B] each [HD, S]
    ropeA = np.concatenate([fc[:, :, 0, 0].T, fc[:, :, 1, 1].T], axis=0)
    ropeBsw = np.concatenate([fc[:, :, 1, 0].T, fc[:, :, 0, 1].T], axis=0)
    rope = np.stack([ropeA, ropeBsw]).astype(BF16)  # [2, HD, S]

    # strict upper triangle masked: key i > query c
    mtri = np.where(np.arange(128)[:, None] > np.arange(128)[None, :],
                    np.float32(NEG), np.float32(0.0)).astype(np.float32)

    xT = [np.ascontiguousarray(x[b].T).astype(BF16) for b in range(B)]
    woutT_f = Wout.T.astype(np.float32)                  # [H(in), H(out)]
    wout_slices = [np.ascontiguousarray(
        woutT_f[:, g * 512:(g + 1) * 512]).astype(BF16) for g in range(4)]

    in_maps = []
    for c in range(NCORES):
        b, g = divmod(c, 4)
        qk_rows = []
        v_rows = []
        for j in range(NHL):
            hh = (g * NHL + j) * HD
            qk_rows.append(Wqkv[0 * H + hh:0 * H + hh + HD][perm] * SCALE)
            qk_rows.append(Wqkv[1 * H + hh:1 * H + hh + HD][perm])
            v_rows.append(Wqkv[2 * H + hh:2 * H + hh + HD])
        # block-major: [8 blocks, H, 128], block i = rows of (head i//2,
        # q if i%2==0 else k)
        wqk = np.stack([r.T for r in qk_rows])           # [8, H, 128]
        wv = np.concatenate(v_rows, axis=0)              # [512, H]
        in_maps.append({
            "xT": xT[b],
            "wqkT": np.ascontiguousarray(wqk).astype(BF16),
            "wvT": np.ascontiguousarray(wv.T).astype(BF16),
            "rope": rope,
            "mtri": mtri,
            "woutT": wout_slices[g],
        })
    return in_maps


def _install_ntff_hook():
    """The image's antenv lacks axon_hooks; shim it so trace=True works."""
    import sys
    import types
    import ctypes
    import contextlib
    if "antenv.axon_hooks" in sys.modules:
        return
    mod = types.ModuleType("antenv.axon_hooks")
    _reg = {"hook": None}
    mod.set_axon_ntff_profile_hook = lambda h: _reg.__setitem__("hook", h)
    mod.get_axon_ntff_profile_hook = lambda: _reg["hook"]
    sys.modules["antenv.axon_hooks"] = mod

    so_path = "/opt/axon/libaxon_pjrt.so"
    try:
        lib = ctypes.CDLL(so_path)
        if not hasattr(lib, "axon_start_nrt_profile"):
            return
        lib.axon_start_nrt_profile.argtypes = [
            ctypes.POINTER(ctypes.c_int64), ctypes.c_size_t]
        lib.axon_start_nrt_profile.restype = ctypes.c_int64
        lib.axon_stop_nrt_profile.argtypes = [ctypes.c_char_p]
        lib.axon_stop_nrt_profile.restype = ctypes.c_int64

        @contextlib.contextmanager
        def _hook(output_dir, device_ids):
            import jax
            jax.devices()
            if device_ids:
                ids = (ctypes.c_int64 * len(device_ids))(*device_ids)
                rc = lib.axon_start_nrt_profile(ids, len(device_ids))
            else:
                rc = lib.axon_start_nrt_profile(None, 0)
            if rc != 0:
                raise RuntimeError(f"axon_start_nrt_profile rc={rc}")
            try:
                yield
            finally:
                n = lib.axon_stop_nrt_profile(str(output_dir).encode())
                print(f"profile: {n} file(s) written to {output_dir}")

        mod.set_axon_ntff_profile_hook(_hook)
    except OSError:
        pass


def _run(in_maps, trace=False):
    if trace:
        _install_ntff_hook()
    from concourse.bass_utils import run_bass_kernel_spmd
    if "nc" not in _cache:
        _cache["nc"] = _build()
    return run_bass_kernel_spmd(_cache["nc"], in_maps,
                                list(range(NCORES)), trace=trace)


def _assemble(r):
    out = np.empty((B, S, H), dtype=np.float32)
    for c in range(NCORES):
        b, g = divmod(c, 4)
        out[b, :, g * 512:(g + 1) * 512] = r.results[c]["out"]
    return out


def kernel(x, attention_mask, frequency_cis, Wqkv, Wout):
    in_maps = _host_prep(x, attention_mask, frequency_cis, Wqkv, Wout)
    r = _run(in_maps)
    return _assemble(r)


def kernel_traced(x, attention_mask, frequency_cis, Wqkv, Wout):
    """Like kernel() but also returns (out, exec_time_ns)."""
    in_maps = _host_prep(x, attention_mask, frequency_cis, Wqkv, Wout)
    r = _run(in_maps, trace=True)
    return _assemble(r), getattr(r, "exec_time_ns", None)


# revision 68
# speedup vs baseline: 1.0022x; 1.0022x over previous
"""Distributed Trainium2 Bass kernel for an attention block (fused, v4).

Reference math (B=2, S=2048, H=2048, NH=16, HD=128):
  qkv = x @ Wqkv.T -> split q,k,v per head -> RoPE(q,k via frequency_cis 2x2)
  scores = (q @ k.T) * 1/sqrt(HD) + causal mask -> softmax -> @ v -> @ Wout.T

Sharding (8 cores): core c handles batch b=c//4 and heads 4*(c%4)..4*(c%4)+3.

v4 changes over v3 (476us):
  - l (softmax denominator) no longer costs a 128x1x512 PE matmul per key
    chunk (~50us of PE): et chunks are summed element-wise on the DVE
    (partition-sum commutes with the chunk-sum), then ONE ones[128,128]
    stationary matmul per (head, slab) yields l already broadcast across
    partitions; 1/l via reciprocal_approx_fast; the normalization multiply
    reads pv straight from PSUM (pvs copies, gpsimd partition_broadcasts,
    and the scalar Ln/Exp chain all deleted).
  - AllGathers are per (slab, head) [128 rows each] and fire as soon as
    that head's at-DMA lands, so the final exposed AG is 1/4 the bytes.
  - out_proj(0)/(1)/(2) are interleaved into attention(2)/proj(3)/
    attention(3) as mq-major chains, so the post-attention tail is only
    AG(3,h3) + the head-major out_proj(3).
  - prologue DMAs strictly in first-consumption order (wqk0/1, all of
    x-slab0, rope, then the rest) to cut PE starvation at start.
"""

import numpy as np
import ml_dtypes
from contextlib import ExitStack

B, S, H, NH, HD = 2, 2048, 2048, 16, 128
NHL = 4          # heads per core
NCORES = 8
SCALE = 1.0 / np.sqrt(HD)
BF16 = ml_dtypes.bfloat16
NEG = -1e9

_cache = {}


def _build():
    import concourse.bass as bass
    import concourse.tile as tile
    from concourse import bacc, mybir
    dt = mybir.dt
    nc = bacc.Bacc("TRN2", target_bir_lowering=False, debug=False,
                   num_devices=NCORES)

    P = 128
    KO = H // P           # 16 contraction chunks for the projections
    NQT = S // 512        # 4 q tiles of 512

    xT = nc.dram_tensor("xT", [H, S], dt.bfloat16, kind="ExternalInput").ap()
    # block-major: wqkT[i] = columns [128i, 128i+128) of Wqk.T, contiguous
    # so the per-matmul-group prologue loads are single fast DMAs
    wqkT = nc.dram_tensor("wqkT", [2 * NHL, H, HD], dt.bfloat16,
                          kind="ExternalInput").ap()
    wvT = nc.dram_tensor("wvT", [H, NHL * HD], dt.bfloat16,
                         kind="ExternalInput").ap()
    rope = nc.dram_tensor("rope", [2, HD, S], dt.bfloat16,
                          kind="ExternalInput").ap()
    mtri = nc.dram_tensor("mtri", [P, P], dt.float32,
                          kind="ExternalInput").ap()
    woutT = nc.dram_tensor("woutT", [H, 512], dt.bfloat16,
                           kind="ExternalInput").ap()
    out_ext = nc.dram_tensor("out", [S, 512], dt.float32,
                             kind="ExternalOutput").ap()

    # internal DRAM for the AllGathers. One AG per slab for slabs 0-2
    # (fully hidden, and each AG carries ~5-7us of serialized CC latency so
    # fewer is better); slab 3 splits A=heads 0-2 / B=head 3 so the tail
    # exposes only the small B gather.
    atl = nc.dram_tensor("attnT_loc", [NQT, NHL, P, 512], dt.bfloat16)
    ats = nc.dram_tensor("attnT_sh", [NQT, 4 * NHL, P, 512], dt.bfloat16)

    with tile.TileContext(nc) as tc, ExitStack() as ctx:
        per = ctx.enter_context(tc.tile_pool(name="per", bufs=1))
        sb = ctx.enter_context(tc.tile_pool(name="sb", bufs=1))
        ps = ctx.enter_context(tc.tile_pool(name="ps", bufs=1, space="PSUM"))

        # persistent: roped q/k in [HD, h, S]; v natural [s%P, s//P, h*HD+d]
        qsb = per.tile([P, NHL, S], dt.bfloat16, tag="qsb")
        ksb = per.tile([P, NHL, S], dt.bfloat16, tag="ksb")
        vsb = per.tile([P, KO, NHL * HD], dt.bfloat16, tag="vsb")
        ones = per.tile([P, P], dt.bfloat16, tag="ones")
        nc.vector.memset(ones[:], 1.0)
        # warm the PE's HAM clock gate with dummy matmuls while the first
        # weight/x DMAs are in flight: activity from ~t=0 flips the 4/8
        # throttle ~2us before the first real matmul would on its own
        warm = ps.tile([P, 512], dt.float32, tag="psc", bufs=4, name="warm")
        for _ in range(24):
            nc.tensor.matmul(warm[:, 0:P], ones[:], ones[:],
                             start=True, stop=True)

        # ---- prologue loads in strict first-consumption order: the first
        # matmul group needs wqk block 0 + x chunks (streamed), then rope
        # for its rope stage, then the remaining weight blocks ----
        xTr = xT.rearrange("(ko p) s -> p ko s", p=P)
        wqkr = wqkT.rearrange("b (ko p) m -> b p ko m", p=P)
        wqk = per.tile([P, 2 * NHL, KO, HD], dt.bfloat16, tag="wqk")
        xn0 = sb.tile([P, KO, 512], dt.bfloat16, tag="xn", bufs=2)
        rsb0 = sb.tile([P, 2, 512], dt.bfloat16, tag="rsb", bufs=2)
        wv = per.tile([P, KO, NHL * HD], dt.bfloat16, tag="wv")
        msb = per.tile([P, P], dt.float32, tag="msb")
        nc.sync.dma_start(msb[:], mtri)   # tiny; warms the DMA path
        # micro-stage the first weight block: the first matmul needs only
        # kc 0-1 (64KB), not the whole 0.5MB block
        nc.sync.dma_start(wqk[:, 0, 0:2], wqkr[0][:, 0:2])
        nc.sync.dma_start(wqk[:, 0, 2:], wqkr[0][:, 2:])
        nc.sync.dma_start(wqk[:, 1], wqkr[1])
        for i, ck in enumerate([(0, 2), (2, 4), (4, 8), (8, 12), (12, 16)]):
            nc.sync.dma_start(xn0[:, slice(*ck), :], xTr[:, slice(*ck), 0:512])
            if i == 1:   # rope is consumed right after the first qk group
                nc.sync.dma_start(
                    rsb0[:], rope.rearrange("r p s -> p r s")[:, :, 0:512])
        for i in range(2, 8):
            nc.sync.dma_start(wqk[:, i], wqkr[i])
        nc.sync.dma_start(wv[:], wvT.rearrange("(ko p) m -> p ko m", p=P))
        wo = per.tile([P, KO, 512], dt.bfloat16, tag="wo")
        nc.sync.dma_start(wo[:], woutT.rearrange("(ko p) n -> p ko n", p=P))

        def proj(n, xn, rsb, interleave=None):
            ns = slice(n * 512, (n + 1) * 512)
            for h in range(NHL):
                for t in range(2):   # q, k with RoPE
                    pp = ps.tile([P, 512], dt.float32, tag="pmm", bufs=2,
                                 name="pp")
                    for kc in range(KO):
                        nc.tensor.matmul(
                            pp[:], wqk[:, h * 2 + t, kc, :], xn[:, kc, :],
                            start=(kc == 0), stop=(kc == KO - 1))
                    # rope input holds [A, swap(B)]; u = q*swap(B), then
                    # DMA-swap u's partition halves so t2 = swap(q)*B,
                    # and dst = q*A + t2.
                    dst = qsb if t == 0 else ksb
                    t1 = sb.tile([P, 512], dt.bfloat16, tag="t1", bufs=2)
                    u = sb.tile([P, 512], dt.bfloat16, tag="u", bufs=2)
                    t2 = sb.tile([P, 512], dt.bfloat16, tag="t2", bufs=2)
                    nc.vector.tensor_tensor(t1[:], pp[:], rsb[:, 0, :],
                                            mybir.AluOpType.mult)
                    nc.vector.tensor_tensor(u[:], pp[:], rsb[:, 1, :],
                                            mybir.AluOpType.mult)
                    # swaps issued from Scalar (idle during proj): gpsimd
                    # blocks on collectives, and sync's at/out-DMAs would
                    # head-of-line-block these behind the finish chain
                    nc.scalar.dma_start(t2[:64], u[64:, :])
                    nc.scalar.dma_start(t2[64:], u[:64, :])
                    nc.vector.tensor_tensor(dst[:, h, ns], t1[:], t2[:],
                                            mybir.AluOpType.add)
                if interleave is not None and h % 2 == 1:
                    # slot an out_proj mq-chain pair between qk head groups
                    interleave(2 * (h // 2))
                    interleave(2 * (h // 2) + 1)
            # v in natural layout: stationary = x s-cols, moving = Wv.
            # evacuation on Scalar (idle during proj) keeps the DVE FIFO
            # clear for the rope chain.
            for j in range(4):
                # slab 0 borrows the idle attention psum so the v matmuls
                # don't contend with the rope-gated pmm banks at startup
                tag, bufs = ("psc", 4) if n == 0 else ("pmm", 2)
                pv = ps.tile([P, 512], dt.float32, tag=tag, bufs=bufs,
                             name="pv")
                for kc in range(KO):
                    nc.tensor.matmul(
                        pv[:], xn[:, kc, j * P:(j + 1) * P], wv[:, kc, :],
                        start=(kc == 0), stop=(kc == KO - 1))
                nc.scalar.copy(vsb[:, n * 4 + j, :], pv[:])

        asb_tiles = {}   # (qt, h) -> sbuf tile [P, 4, 512]; layout [p, g, s]
        deferred_pf = []

        def gather(qt, h):
            """AllGather one (slab, head) [128 rows] + prefetch. Per-head
            gathers keep each CC mesh step's latency (~10-15us serialized on
            the CC core) small and early; the last slab exposes only h3's.

            For slab 3 heads 0-1 the COLLECTIVE fires immediately (the AG
            pipeline start is the tail's binding constraint) but the
            prefetch DMA emission is deferred until head 2's gather, after
            the last out_proj(1) chain has been traced: the prefetch
            overwrites slab 1's buffer slot, and tile's WAR tracking only
            orders it after already-emitted readers."""
            nc.gpsimd.collective_compute(
                "AllGather",
                mybir.AluOpType.bypass,
                replica_groups=[[0, 1, 2, 3], [4, 5, 6, 7]],
                ins=[atl.ap()[qt, h].opt()],
                outs=[ats.ap()[qt, 4 * h:4 * h + 4].opt()],
            )
            # prefetch on gpsimd: the load waits on the AG semaphore, and a
            # sync-queue wait would head-of-line-block the rope swaps and
            # x loads queued behind it. The very last gather's prefetch
            # rides the scalar queue instead (idle after the final EXP) -
            # it is on the critical tail path and skips the gpsimd wake.
            asb = sb.tile([P, 4, 512], dt.bfloat16, tag=f"asb{h}", bufs=2,
                          name="asbh")
            src = ats.ap()[qt, 4 * h:4 * h + 4].rearrange("k p s -> p k s")
            if qt == NQT - 1 and h < 2:
                deferred_pf.append((asb, src))
            else:
                if qt == NQT - 1 and h == 2:
                    for a, s in deferred_pf:
                        nc.gpsimd.dma_start(a[:], s)
                    deferred_pf.clear()
                eng = (nc.scalar if (qt == NQT - 1 and h == NHL - 1)
                       else nc.gpsimd)
                eng.dma_start(asb[:], src)
            asb_tiles[(qt, h)] = asb

        def attention(qt, interleave=None, pre_gather=None):
            for h in range(NHL):
                nkc = 4 * qt + 4
                ppv = ps.tile([P, 512], dt.float32, tag="ppv", bufs=2,
                              name="pvacc")
                etacc = sb.tile([P, 512], dt.bfloat16, tag="etacc", bufs=2)
                emitted = []

                def flush_one():
                    kc, et, q0 = emitted.pop(0)
                    nc.tensor.matmul(ppv[:, q0:],
                                     vsb[:, kc, h * P:(h + 1) * P],
                                     et[:, q0:], start=(kc == 0),
                                     stop=(kc == nkc - 1))

                for kc in range(nkc):
                    # columns < q0 of this chunk are fully causally masked:
                    # restrict every op to the active [q0:] range (the
                    # skipped region contributes exact zeros to l and pv)
                    j = kc - 4 * qt
                    q0 = j * P if j > 0 else 0
                    sc = ps.tile([P, 512], dt.float32, tag="psc", bufs=4,
                                 name="sc")
                    nc.tensor.matmul(sc[:, q0:],
                                     ksb[:, h, kc * P:(kc + 1) * P],
                                     qsb[:, h, qt * 512 + q0:
                                         (qt + 1) * 512],
                                     start=True, stop=True)
                    et = sb.tile([P, 512], dt.bfloat16, tag="et", bufs=5)
                    if j >= 0:   # straddles the causal diagonal
                        nc.vector.tensor_tensor(
                            sc[:, j * P:(j + 1) * P],
                            sc[:, j * P:(j + 1) * P],
                            msb[:], mybir.AluOpType.add)
                    nc.scalar.activation(
                        et[:, q0:], sc[:, q0:],
                        mybir.ActivationFunctionType.Exp)
                    # DVE partial-sum for l: summing chunks element-wise
                    # commutes with the later partition sum
                    if kc == 0:
                        nc.vector.tensor_copy(etacc[:], et[:])
                    else:
                        nc.vector.tensor_tensor(etacc[:, q0:],
                                                etacc[:, q0:], et[:, q0:],
                                                mybir.AluOpType.add)
                    emitted.append((kc, et, q0))
                    while len(emitted) > 3:
                        flush_one()
                while emitted:
                    flush_one()

                # l broadcast across partitions via ones[128,128] stationary;
                # its psum comes from the psc rotation (freed bank -> bufs=4)
                lps = ps.tile([P, 512], dt.float32, tag="psc", bufs=4,
                              name="lacc")
                nc.tensor.matmul(lps[:], ones[:], etacc[:],
                                 start=True, stop=True)
                rl = sb.tile([P, 512], dt.float32, tag="rl", bufs=2)
                nc.vector.reciprocal_approx_fast(rl[:], lps[:])
                at = sb.tile([P, 512], dt.bfloat16, tag="at", bufs=2)
                nc.vector.tensor_tensor(at[:], ppv[:], rl[:],
                                        mybir.AluOpType.mult)
                nc.sync.dma_start(atl.ap()[qt, h], at[:])
                # pre_gather thunks run BEFORE this head's gather is
                # emitted: their asb reads must be traced before the
                # slab-3 prefetches overwrite the shared buffer slots
                # (tile's WAR tracking only sees already-emitted readers)
                for fn in (pre_gather or {}).get(h, []):
                    fn()
                gather(qt, h)
                for fn in (interleave or {}).get(h, []):
                    fn()

        def mk_outproj_chain(qt):
            """Returns chain(mq): one mq's full 16-MM accumulation +
            evacuation for out_proj(qt), usable as an interleave slot."""
            def chain(mq):
                po = ps.tile([P, 512], dt.float32, tag="pmm", bufs=2,
                             name="po")
                i = 0
                for h in range(NHL):
                    a = asb_tiles[(qt, h)]
                    for g in range(4):
                        nc.tensor.matmul(
                            po[:], a[:, g, mq * P:(mq + 1) * P],
                            wo[:, g * 4 + h, :],
                            start=(i == 0), stop=(i == 15))
                        i += 1
                # evacuation on DVE: a scalar-queue copy here would wait on
                # this chain's matmuls while EXPs of the surrounding
                # attention queue behind it (strict FIFO) - that stall
                # cascades into late at-DMAs and late AllGathers
                ev = sb.tile([P, 512], dt.float32, tag="ev", bufs=2)
                nc.vector.tensor_copy(ev[:], po[:])
                nc.sync.dma_start(
                    out_ext[(qt * 4 + mq) * P:(qt * 4 + mq + 1) * P, :],
                    ev[:])
            return chain

        def out_proj_final(qt):
            # all 4 accumulators live; consume the B part (head 3, early
            # gather) first, then the A heads as the big gather lands
            pos = []
            for mq in range(4):
                tag, bufs = (("pmm", 2) if mq < 2 else ("psc", 4))
                pos.append(ps.tile([P, 512], dt.float32, tag=tag,
                                   bufs=bufs, name="pof"))
            for h in range(NHL):
                a = asb_tiles[(qt, h)]
                for mq in range(4):
                    for g in range(4):
                        nc.tensor.matmul(
                            pos[mq][:], a[:, g, mq * P:(mq + 1) * P],
                            wo[:, g * 4 + h, :],
                            start=(h == 0 and g == 0),
                            stop=(h == NHL - 1 and g == 3))
                    if h == NHL - 1:
                        # evacuate each mq as soon as its chain stops,
                        # alternating scalar/vector so two copy+DMA
                        # chains drain in parallel at the very end
                        ev = sb.tile([P, 512], dt.float32, tag="ev",
                                     bufs=2)
                        orow = out_ext[(qt * 4 + mq) * P:
                                       (qt * 4 + mq + 1) * P, :]
                        if mq % 2 == 0:
                            nc.scalar.copy(ev[:], pos[mq][:])
                            nc.scalar.dma_start(orow, ev[:])
                        else:
                            nc.vector.tensor_copy(ev[:], pos[mq][:])
                            nc.sync.dma_start(orow, ev[:])

        from functools import partial
        rr = rope.rearrange("r p s -> p r s")
        xn_t, rsb_t = xn0, rsb0
        for n in range(NQT):
            # issue next slab's x/rope loads first: with xn double-buffered
            # they overlap all of proj(n)+attention(n) instead of starting
            # after proj(n) finished reading the shared buffer
            if n + 1 < NQT:
                ns2 = slice((n + 1) * 512, (n + 2) * 512)
                xn_nx = sb.tile([P, KO, 512], dt.bfloat16, tag="xn",
                                bufs=2, name="xn_n")
                nc.sync.dma_start(xn_nx[:], xTr[:, :, ns2])
                rsb_nx = sb.tile([P, 2, 512], dt.bfloat16, tag="rsb",
                                 bufs=2, name="rsb_n")
                nc.sync.dma_start(rsb_nx[:], rr[:, :, ns2])
            proj(n, xn_t, rsb_t)
            il, pg = None, None
            if n == 2:
                c0 = mk_outproj_chain(0)
                il = {h: [partial(c0, h)] for h in range(NHL)}
            elif n == 3:
                # out_proj(1) chains spread across heads 0-2 (emitted as
                # pre-gather thunks, before each head's own pf): the AG
                # inputs for h1/h2 then land earlier relative to the
                # slab's end and the serial ~14us/gather CC drain delivers
                # the last gather ~T+18 instead of ~T+27
                c1 = mk_outproj_chain(1)
                pg = {0: [partial(c1, 0), partial(c1, 1)],
                      1: [partial(c1, 2)], 2: [partial(c1, 3)]}
            attention(n, interleave=il, pre_gather=pg)
            if n + 1 < NQT:
                xn_t, rsb_t = xn_nx, rsb_nx
        # slab 3's per-head AllGathers drain serially on the CC core at
        # ~12-14us each from the first at-DMA; attention(3) runs pure (so
        # the gather inputs land as early as possible) and out_proj(2) +
        # OPF's h0-h2 groups fill until h3's gather lands
        c2 = mk_outproj_chain(2)
        for mq in range(4):
            c2(mq)
        out_proj_final(NQT - 1)

    nc.compile()
    return nc


def _host_prep(x, attention_mask, frequency_cis, Wqkv, Wout):
    """Build the 8 per-core input maps (numpy only)."""
    x = np.asarray(x, dtype=np.float32)
    fc = np.asarray(frequency_cis, dtype=np.float32)
    Wqkv = np.asarray(Wqkv, dtype=np.float32)
    Wout = np.asarray(Wout, dtype=np.float32)

    # rotate-half permutation of the head dim: new row p<64 <- old 2p,
    # p>=64 <- old 2(p-64)+1
    perm = np.concatenate([np.arange(0, HD, 2), np.arange(1, HD, 2)])
    # rope coefficients in permuted layout: [A;B] each [HD, S]
    ropeA = np.concatenate([fc[:, :, 0, 0].T, fc[:, :, 1, 1].T], axis=0)
    ropeBsw = np.concatenate([fc[:, :, 1, 0].T, fc[:, :, 0, 1].T], axis=0)
    rope = np.stack([ropeA, ropeBsw]).astype(BF16)  # [2, HD, S]

    # strict upper triangle masked: key i > query c
    mtri = np.where(np.arange(128)[:, None] > np.arange(128)[None, :],
                    np.float32(NEG), np.float32(0.0)).astype(np.float32)

    xT = [np.ascontiguousarray(x[b].T).astype(BF16) for b in range(B)]
    woutT_f = Wout.T.astype(np.float32)                  # [H(in), H(out)]
    wout_slices = [np.ascontiguousarray(
        woutT_f[:, g * 512:(g + 1) * 512]).astype(BF16) for g in range(4)]

    in_maps = []
    for c in range(NCORES):
        b, g = divmod(c, 4)
        qk_rows = []
        v_rows = []
        for j in range(NHL):
            hh = (g * NHL + j) * HD
            qk_rows.append(Wqkv[0 * H + hh:0 * H + hh + HD][perm] * SCALE)
            qk_rows.append(Wqkv[1 * H + hh:1 * H + hh + HD][perm])
            v_rows.append(Wqkv[2 * H + hh:2 * H + hh + HD])
        # block-major: [8 blocks, H, 128], block i = rows of (head i//2,
        # q if i%2==0 else k)
        wqk = np.stack([r.T for r in qk_rows])           # [8, H, 128]
        wv = np.concatenate(v_rows, axis=0)              # [512, H]
        in_maps.append({
            "xT": xT[b],
            "wqkT": np.ascontiguousarray(wqk).astype(BF16),
            "wvT": np.ascontiguousarray(wv.T).astype(BF16),
            "rope": rope,
            "mtri": mtri,
            "woutT": wout_slices[g],
        })
    return in_maps


def _install_ntff_hook():
    """The image's antenv lacks axon_hooks; shim it so trace=True works."""
    import sys
    import types
    import ctypes
    import contextlib
    if "antenv.axon_hooks" in sys.modules:
        return
    mod = types.ModuleType("antenv.axon_hooks")
    _reg = {"hook": None}
    mod.set_axon_ntff_profile_hook = lambda h: _reg.__setitem__("hook", h)
    mod.get_axon_ntff_profile_hook = lambda: _reg["hook"]
    sys.modules["antenv.axon_hooks"] = mod

    so_path = "/opt/axon/libaxon_pjrt.so"
    try:
        lib = ctypes.CDLL(so_path)
        if not hasattr(lib, "axon_start_nrt_profile"):
            return
        lib.axon_start_nrt_profile.argtypes = [
            ctypes.POINTER(ctypes.c_int64), ctypes.c_size_t]
        lib.axon_start_nrt_profile.restype = ctypes.c_int64
        lib.axon_stop_nrt_profile.argtypes = [ctypes.c_char_p]
        lib.axon_stop_nrt_profile.restype = ctypes.c_int64

        @contextlib.contextmanager
        def _hook(output_dir, device_ids):
            import jax
            jax.devices()
            if device_ids:
                ids = (ctypes.c_int64 * len(device_ids))(*device_ids)
                rc = lib.axon_start_nrt_profile(ids, len(device_ids))
            else:
                rc = lib.axon_start_nrt_profile(None, 0)
            if rc != 0:
                raise RuntimeError(f"axon_start_nrt_profile rc={rc}")
            try:
                yield
            finally:
                n = lib.axon_stop_nrt_profile(str(output_dir).encode())
                print(f"profile: {n} file(s) written to {output_dir}")

        mod.set_axon_ntff_profile_hook(_hook)
    except OSError:
        pass


def _run(in_maps, trace=False):
    if trace:
        _install_ntff_hook()
    from concourse.bass_utils import run_bass_kernel_spmd
    if "nc" not in _cache:
        _cache["nc"] = _build()
    return run_bass_kernel_spmd(_cache["nc"], in_maps,
                                list(range(NCORES)), trace=trace)


def _assemble(r):
    out = np.empty((B, S, H), dtype=np.float32)
    for c in range(NCORES):
        b, g = divmod(c, 4)
        out[b, :, g * 512:(g + 1) * 512] = r.results[c]["out"]
    return out


def kernel(x, attention_mask, frequency_cis, Wqkv, Wout):
    in_maps = _host_prep(x, attention_mask, frequency_cis, Wqkv, Wout)
    r = _run(in_maps)
    return _assemble(r)


def kernel_traced(x, attention_mask, frequency_cis, Wqkv, Wout):
    """Like kernel() but also returns (out, exec_time_ns)."""
    in_maps = _host_prep(x, attention_mask, frequency_cis, Wqkv, Wout)
    r = _run(in_maps, trace=True)
    return _assemble(r), getattr(r, "exec_time_ns", None)


# revision 69
# speedup vs baseline: 1.0145x; 1.0123x over previous
"""Distributed Trainium2 Bass kernel for an attention block (fused, v4).

Reference math (B=2, S=2048, H=2048, NH=16, HD=128):
  qkv = x @ Wqkv.T -> split q,k,v per head -> RoPE(q,k via frequency_cis 2x2)
  scores = (q @ k.T) * 1/sqrt(HD) + causal mask -> softmax -> @ v -> @ Wout.T

Sharding (8 cores): core c handles batch b=c//4 and heads 4*(c%4)..4*(c%4)+3.

v4 changes over v3 (476us):
  - l (softmax denominator) no longer costs a 128x1x512 PE matmul per key
    chunk (~50us of PE): et chunks are summed element-wise on the DVE
    (partition-sum commutes with the chunk-sum), then ONE ones[128,128]
    stationary matmul per (head, slab) yields l already broadcast across
    partitions; 1/l via reciprocal_approx_fast; the normalization multiply
    reads pv straight from PSUM (pvs copies, gpsimd partition_broadcasts,
    and the scalar Ln/Exp chain all deleted).
  - AllGathers are per (slab, head) [128 rows each] and fire as soon as
    that head's at-DMA lands, so the final exposed AG is 1/4 the bytes.
  - out_proj(0)/(1)/(2) are interleaved into attention(2)/proj(3)/
    attention(3) as mq-major chains, so the post-attention tail is only
    AG(3,h3) + the head-major out_proj(3).
  - prologue DMAs strictly in first-consumption order (wqk0/1, all of
    x-slab0, rope, then the rest) to cut PE starvation at start.
"""

import numpy as np
import ml_dtypes
from contextlib import ExitStack

B, S, H, NH, HD = 2, 2048, 2048, 16, 128
NHL = 4          # heads per core
NCORES = 8
SCALE = 1.0 / np.sqrt(HD)
BF16 = ml_dtypes.bfloat16
NEG = -1e9

_cache = {}


def _build():
    import concourse.bass as bass
    import concourse.tile as tile
    from concourse import bacc, mybir
    dt = mybir.dt
    nc = bacc.Bacc("TRN2", target_bir_lowering=False, debug=False,
                   num_devices=NCORES)

    P = 128
    KO = H // P           # 16 contraction chunks for the projections
    NQT = S // 512        # 4 q tiles of 512

    xT = nc.dram_tensor("xT", [H, S], dt.bfloat16, kind="ExternalInput").ap()
    # block-major: wqkT[i] = columns [128i, 128i+128) of Wqk.T, contiguous
    # so the per-matmul-group prologue loads are single fast DMAs
    wqkT = nc.dram_tensor("wqkT", [2 * NHL, H, HD], dt.bfloat16,
                          kind="ExternalInput").ap()
    wvT = nc.dram_tensor("wvT", [H, NHL * HD], dt.bfloat16,
                         kind="ExternalInput").ap()
    rope = nc.dram_tensor("rope", [2, HD, S], dt.bfloat16,
                          kind="ExternalInput").ap()
    mtri = nc.dram_tensor("mtri", [P, P], dt.float32,
                          kind="ExternalInput").ap()
    woutT = nc.dram_tensor("woutT", [H, 512], dt.bfloat16,
                           kind="ExternalInput").ap()
    out_ext = nc.dram_tensor("out", [S, 512], dt.float32,
                             kind="ExternalOutput").ap()

    # internal DRAM for the AllGathers. One AG per slab for slabs 0-2
    # (fully hidden, and each AG carries ~5-7us of serialized CC latency so
    # fewer is better); slab 3 splits A=heads 0-2 / B=head 3 so the tail
    # exposes only the small B gather.
    atl = nc.dram_tensor("attnT_loc", [NQT, NHL, P, 512], dt.bfloat16)
    ats = nc.dram_tensor("attnT_sh", [NQT, 4 * NHL, P, 512], dt.bfloat16)

    with tile.TileContext(nc) as tc, ExitStack() as ctx:
        per = ctx.enter_context(tc.tile_pool(name="per", bufs=1))
        sb = ctx.enter_context(tc.tile_pool(name="sb", bufs=1))
        ps = ctx.enter_context(tc.tile_pool(name="ps", bufs=1, space="PSUM"))

        # persistent: roped q/k in [HD, h, S]; v natural [s%P, s//P, h*HD+d]
        qsb = per.tile([P, NHL, S], dt.bfloat16, tag="qsb")
        ksb = per.tile([P, NHL, S], dt.bfloat16, tag="ksb")
        vsb = per.tile([P, KO, NHL * HD], dt.bfloat16, tag="vsb")
        ones = per.tile([P, P], dt.bfloat16, tag="ones")
        nc.vector.memset(ones[:], 1.0)
        # warm the PE's HAM clock gate with dummy matmuls while the first
        # weight/x DMAs are in flight: activity from ~t=0 flips the 4/8
        # throttle ~2us before the first real matmul would on its own
        warm = ps.tile([P, 512], dt.float32, tag="psc", bufs=4, name="warm")
        for _ in range(64):
            nc.tensor.matmul(warm[:, 0:P], ones[:], ones[:],
                             start=True, stop=True)

        # ---- prologue loads in strict first-consumption order: the first
        # matmul group needs wqk block 0 + x chunks (streamed), then rope
        # for its rope stage, then the remaining weight blocks ----
        xTr = xT.rearrange("(ko p) s -> p ko s", p=P)
        wqkr = wqkT.rearrange("b (ko p) m -> b p ko m", p=P)
        wqk = per.tile([P, 2 * NHL, KO, HD], dt.bfloat16, tag="wqk")
        xn0 = sb.tile([P, KO, 512], dt.bfloat16, tag="xn", bufs=2)
        rsb0 = sb.tile([P, 2, 512], dt.bfloat16, tag="rsb", bufs=2)
        wv = per.tile([P, KO, NHL * HD], dt.bfloat16, tag="wv")
        msb = per.tile([P, P], dt.float32, tag="msb")
        nc.sync.dma_start(msb[:], mtri)   # tiny; warms the DMA path
        # micro-stage the first weight block: the first matmul needs only
        # kc 0-1 (64KB), not the whole 0.5MB block
        nc.sync.dma_start(wqk[:, 0, 0:2], wqkr[0][:, 0:2])
        nc.sync.dma_start(wqk[:, 0, 2:], wqkr[0][:, 2:])
        nc.sync.dma_start(wqk[:, 1], wqkr[1])
        for i, ck in enumerate([(0, 2), (2, 4), (4, 8), (8, 12), (12, 16)]):
            nc.sync.dma_start(xn0[:, slice(*ck), :], xTr[:, slice(*ck), 0:512])
            if i == 1:   # rope is consumed right after the first qk group
                nc.sync.dma_start(
                    rsb0[:], rope.rearrange("r p s -> p r s")[:, :, 0:512])
        for i in range(2, 8):
            nc.sync.dma_start(wqk[:, i], wqkr[i])
        nc.sync.dma_start(wv[:], wvT.rearrange("(ko p) m -> p ko m", p=P))
        wo = per.tile([P, KO, 512], dt.bfloat16, tag="wo")
        nc.sync.dma_start(wo[:], woutT.rearrange("(ko p) n -> p ko n", p=P))

        def proj(n, xn, rsb, interleave=None):
            ns = slice(n * 512, (n + 1) * 512)
            for h in range(NHL):
                for t in range(2):   # q, k with RoPE
                    pp = ps.tile([P, 512], dt.float32, tag="pmm", bufs=2,
                                 name="pp")
                    for kc in range(KO):
                        nc.tensor.matmul(
                            pp[:], wqk[:, h * 2 + t, kc, :], xn[:, kc, :],
                            start=(kc == 0), stop=(kc == KO - 1))
                    # rope input holds [A, swap(B)]; u = q*swap(B), then
                    # DMA-swap u's partition halves so t2 = swap(q)*B,
                    # and dst = q*A + t2.
                    dst = qsb if t == 0 else ksb
                    t1 = sb.tile([P, 512], dt.bfloat16, tag="t1", bufs=2)
                    u = sb.tile([P, 512], dt.bfloat16, tag="u", bufs=2)
                    t2 = sb.tile([P, 512], dt.bfloat16, tag="t2", bufs=2)
                    nc.vector.tensor_tensor(t1[:], pp[:], rsb[:, 0, :],
                                            mybir.AluOpType.mult)
                    nc.vector.tensor_tensor(u[:], pp[:], rsb[:, 1, :],
                                            mybir.AluOpType.mult)
                    # swaps issued from Scalar (idle during proj): gpsimd
                    # blocks on collectives, and sync's at/out-DMAs would
                    # head-of-line-block these behind the finish chain
                    nc.scalar.dma_start(t2[:64], u[64:, :])
                    nc.scalar.dma_start(t2[64:], u[:64, :])
                    nc.vector.tensor_tensor(dst[:, h, ns], t1[:], t2[:],
                                            mybir.AluOpType.add)
                if interleave is not None and h % 2 == 1:
                    # slot an out_proj mq-chain pair between qk head groups
                    interleave(2 * (h // 2))
                    interleave(2 * (h // 2) + 1)
            # v in natural layout: stationary = x s-cols, moving = Wv.
            # evacuation on Scalar (idle during proj) keeps the DVE FIFO
            # clear for the rope chain.
            for j in range(4):
                # slab 0 borrows the idle attention psum so the v matmuls
                # don't contend with the rope-gated pmm banks at startup
                tag, bufs = ("psc", 4) if n == 0 else ("pmm", 2)
                pv = ps.tile([P, 512], dt.float32, tag=tag, bufs=bufs,
                             name="pv")
                for kc in range(KO):
                    nc.tensor.matmul(
                        pv[:], xn[:, kc, j * P:(j + 1) * P], wv[:, kc, :],
                        start=(kc == 0), stop=(kc == KO - 1))
                nc.scalar.copy(vsb[:, n * 4 + j, :], pv[:])

        asb_tiles = {}   # (qt, h) -> sbuf tile [P, 4, 512]; layout [p, g, s]
        deferred_pf = []

        def gather(qt, h):
            """AllGather one (slab, head) [128 rows] + prefetch. Per-head
            gathers keep each CC mesh step's latency (~10-15us serialized on
            the CC core) small and early; the last slab exposes only h3's.

            For slab 3 heads 0-1 the COLLECTIVE fires immediately (the AG
            pipeline start is the tail's binding constraint) but the
            prefetch DMA emission is deferred until head 2's gather, after
            the last out_proj(1) chain has been traced: the prefetch
            overwrites slab 1's buffer slot, and tile's WAR tracking only
            orders it after already-emitted readers."""
            nc.gpsimd.collective_compute(
                "AllGather",
                mybir.AluOpType.bypass,
                replica_groups=[[0, 1, 2, 3], [4, 5, 6, 7]],
                ins=[atl.ap()[qt, h].opt()],
                outs=[ats.ap()[qt, 4 * h:4 * h + 4].opt()],
            )
            # prefetch on gpsimd: the load waits on the AG semaphore, and a
            # sync-queue wait would head-of-line-block the rope swaps and
            # x loads queued behind it. The very last gather's prefetch
            # rides the scalar queue instead (idle after the final EXP) -
            # it is on the critical tail path and skips the gpsimd wake.
            asb = sb.tile([P, 4, 512], dt.bfloat16, tag=f"asb{h}", bufs=2,
                          name="asbh")
            src = ats.ap()[qt, 4 * h:4 * h + 4].rearrange("k p s -> p k s")
            if qt == NQT - 1 and h < 2:
                deferred_pf.append((asb, src))
            else:
                if qt == NQT - 1 and h == 2:
                    for a, s in deferred_pf:
                        nc.gpsimd.dma_start(a[:], s)
                    deferred_pf.clear()
                eng = (nc.scalar if (qt == NQT - 1 and h == NHL - 1)
                       else nc.gpsimd)
                eng.dma_start(asb[:], src)
            asb_tiles[(qt, h)] = asb

        def attention(qt, interleave=None, pre_gather=None):
            for h in range(NHL):
                nkc = 4 * qt + 4
                ppv = ps.tile([P, 512], dt.float32, tag="ppv", bufs=2,
                              name="pvacc")
                etacc = sb.tile([P, 512], dt.bfloat16, tag="etacc", bufs=2)
                emitted = []

                def flush_one():
                    kc, et, q0 = emitted.pop(0)
                    nc.tensor.matmul(ppv[:, q0:],
                                     vsb[:, kc, h * P:(h + 1) * P],
                                     et[:, q0:], start=(kc == 0),
                                     stop=(kc == nkc - 1))

                for kc in range(nkc):
                    # columns < q0 of this chunk are fully causally masked:
                    # restrict every op to the active [q0:] range (the
                    # skipped region contributes exact zeros to l and pv)
                    j = kc - 4 * qt
                    q0 = j * P if j > 0 else 0
                    sc = ps.tile([P, 512], dt.float32, tag="psc", bufs=4,
                                 name="sc")
                    nc.tensor.matmul(sc[:, q0:],
                                     ksb[:, h, kc * P:(kc + 1) * P],
                                     qsb[:, h, qt * 512 + q0:
                                         (qt + 1) * 512],
                                     start=True, stop=True)
                    et = sb.tile([P, 512], dt.bfloat16, tag="et", bufs=5)
                    if j >= 0:   # straddles the causal diagonal
                        nc.vector.tensor_tensor(
                            sc[:, j * P:(j + 1) * P],
                            sc[:, j * P:(j + 1) * P],
                            msb[:], mybir.AluOpType.add)
                    nc.scalar.activation(
                        et[:, q0:], sc[:, q0:],
                        mybir.ActivationFunctionType.Exp)
                    # DVE partial-sum for l: summing chunks element-wise
                    # commutes with the later partition sum
                    if kc == 0:
                        nc.vector.tensor_copy(etacc[:], et[:])
                    else:
                        nc.vector.tensor_tensor(etacc[:, q0:],
                                                etacc[:, q0:], et[:, q0:],
                                                mybir.AluOpType.add)
                    emitted.append((kc, et, q0))
                    while len(emitted) > 3:
                        flush_one()
                while emitted:
                    flush_one()

                # l broadcast across partitions via ones[128,128] stationary;
                # its psum comes from the psc rotation (freed bank -> bufs=4)
                lps = ps.tile([P, 512], dt.float32, tag="psc", bufs=4,
                              name="lacc")
                nc.tensor.matmul(lps[:], ones[:], etacc[:],
                                 start=True, stop=True)
                rl = sb.tile([P, 512], dt.float32, tag="rl", bufs=2)
                nc.vector.reciprocal_approx_fast(rl[:], lps[:])
                at = sb.tile([P, 512], dt.bfloat16, tag="at", bufs=2)
                nc.vector.tensor_tensor(at[:], ppv[:], rl[:],
                                        mybir.AluOpType.mult)
                nc.sync.dma_start(atl.ap()[qt, h], at[:])
                # pre_gather thunks run BEFORE this head's gather is
                # emitted: their asb reads must be traced before the
                # slab-3 prefetches overwrite the shared buffer slots
                # (tile's WAR tracking only sees already-emitted readers)
                for fn in (pre_gather or {}).get(h, []):
                    fn()
                gather(qt, h)
                for fn in (interleave or {}).get(h, []):
                    fn()

        def mk_outproj_chain(qt):
            """Returns chain(mq): one mq's full 16-MM accumulation +
            evacuation for out_proj(qt), usable as an interleave slot."""
            def chain(mq):
                po = ps.tile([P, 512], dt.float32, tag="pmm", bufs=2,
                             name="po")
                i = 0
                for h in range(NHL):
                    a = asb_tiles[(qt, h)]
                    for g in range(4):
                        nc.tensor.matmul(
                            po[:], a[:, g, mq * P:(mq + 1) * P],
                            wo[:, g * 4 + h, :],
                            start=(i == 0), stop=(i == 15))
                        i += 1
                # evacuation on DVE: a scalar-queue copy here would wait on
                # this chain's matmuls while EXPs of the surrounding
                # attention queue behind it (strict FIFO) - that stall
                # cascades into late at-DMAs and late AllGathers
                ev = sb.tile([P, 512], dt.float32, tag="ev", bufs=2)
                nc.vector.tensor_copy(ev[:], po[:])
                nc.sync.dma_start(
                    out_ext[(qt * 4 + mq) * P:(qt * 4 + mq + 1) * P, :],
                    ev[:])
            return chain

        def out_proj_final(qt):
            # all 4 accumulators live; consume the B part (head 3, early
            # gather) first, then the A heads as the big gather lands
            pos = []
            for mq in range(4):
                tag, bufs = (("pmm", 2) if mq < 2 else ("psc", 4))
                pos.append(ps.tile([P, 512], dt.float32, tag=tag,
                                   bufs=bufs, name="pof"))
            for h in range(NHL):
                a = asb_tiles[(qt, h)]
                for mq in range(4):
                    for g in range(4):
                        nc.tensor.matmul(
                            pos[mq][:], a[:, g, mq * P:(mq + 1) * P],
                            wo[:, g * 4 + h, :],
                            start=(h == 0 and g == 0),
                            stop=(h == NHL - 1 and g == 3))
                    if h == NHL - 1:
                        # evacuate each mq as soon as its chain stops,
                        # alternating scalar/vector so two copy+DMA
                        # chains drain in parallel at the very end
                        ev = sb.tile([P, 512], dt.float32, tag="ev",
                                     bufs=2)
                        orow = out_ext[(qt * 4 + mq) * P:
                                       (qt * 4 + mq + 1) * P, :]
                        if mq % 2 == 0:
                            nc.scalar.copy(ev[:], pos[mq][:])
                            nc.scalar.dma_start(orow, ev[:])
                        else:
                            nc.vector.tensor_copy(ev[:], pos[mq][:])
                            nc.sync.dma_start(orow, ev[:])

        from functools import partial
        rr = rope.rearrange("r p s -> p r s")
        xn_t, rsb_t = xn0, rsb0
        for n in range(NQT):
            # issue next slab's x/rope loads first: with xn double-buffered
            # they overlap all of proj(n)+attention(n) instead of starting
            # after proj(n) finished reading the shared buffer
            if n + 1 < NQT:
                ns2 = slice((n + 1) * 512, (n + 2) * 512)
                xn_nx = sb.tile([P, KO, 512], dt.bfloat16, tag="xn",
                                bufs=2, name="xn_n")
                nc.sync.dma_start(xn_nx[:], xTr[:, :, ns2])
                rsb_nx = sb.tile([P, 2, 512], dt.bfloat16, tag="rsb",
                                 bufs=2, name="rsb_n")
                nc.sync.dma_start(rsb_nx[:], rr[:, :, ns2])
            proj(n, xn_t, rsb_t)
            il, pg = None, None
            if n == 2:
                c0 = mk_outproj_chain(0)
                il = {h: [partial(c0, h)] for h in range(NHL)}
            elif n == 3:
                # out_proj(1) chains spread across heads 0-2 (emitted as
                # pre-gather thunks, before each head's own pf): the AG
                # inputs for h1/h2 then land earlier relative to the
                # slab's end and the serial ~14us/gather CC drain delivers
                # the last gather ~T+18 instead of ~T+27
                c1 = mk_outproj_chain(1)
                pg = {0: [partial(c1, 0), partial(c1, 1)],
                      1: [partial(c1, 2)], 2: [partial(c1, 3)]}
            attention(n, interleave=il, pre_gather=pg)
            if n + 1 < NQT:
                xn_t, rsb_t = xn_nx, rsb_nx
        # slab 3's per-head AllGathers drain serially on the CC core at
        # ~12-14us each from the first at-DMA; attention(3) runs pure (so
        # the gather inputs land as early as possible) and out_proj(2) +
        # OPF's h0-h2 groups fill until h3's gather lands
        c2 = mk_outproj_chain(2)
        for mq in range(4):
            c2(mq)
        out_proj_final(NQT - 1)

    nc.compile()
    return nc


def _host_prep(x, attention_mask, frequency_cis, Wqkv, Wout):
    """Build the 8 per-core input maps (numpy only)."""
    x = np.asarray(x, dtype=np.float32)
    fc = np.asarray(frequency_cis, dtype=np.float32)
    Wqkv = np.asarray(Wqkv, dtype=np.float32)
    Wout = np.asarray(Wout, dtype=np.float32)

    # rotate-half permutation of the head dim: new row p<64 <- old 2p,
    # p>=64 <- old 2(p-64)+1
    perm = np.concatenate([np.arange(0, HD, 2), np.arange(1, HD, 2)])
    # rope coefficients in permuted layout: [A;B] each [HD, S]
    ropeA = np.concatenate([fc[:, :, 0, 0].T, fc[:, :, 1, 1].T], axis=0)
    ropeBsw = np.concatenate([fc[:, :, 1, 0].T, fc[:, :, 0, 1].T], axis=0)
    rope = np.stack([ropeA, ropeBsw]).astype(BF16)  # [2, HD, S]

    # strict upper triangle masked: key i > query c
    mtri = np.where(np.arange(128)[:, None] > np.arange(128)[None, :],
                    np.float32(NEG), np.float32(0.0)).astype(np.float32)

    xT = [np.ascontiguousarray(x[b].T).astype(BF16) for b in range(B)]
    woutT_f = Wout.T.astype(np.float32)                  # [H(in), H(out)]
    wout_slices = [np.ascontiguousarray(
        woutT_f[:, g * 512:(g + 1) * 512]).astype(BF16) for g in range(4)]

    in_maps = []
    for c in range(NCORES):
        b, g = divmod(c, 4)
        qk_rows = []
        v_rows = []
        for j in range(NHL):
            hh = (g * NHL + j) * HD
            qk_rows.append(Wqkv[0 * H + hh:0 * H + hh + HD][perm] * SCALE)
            qk_rows.append(Wqkv[1 * H + hh:1 * H + hh + HD][perm])
            v_rows.append(Wqkv[2 * H + hh:2 * H + hh + HD])
        # block-major: [8 blocks, H, 128], block i = rows of (head i//2,
        # q if i%2==0 else k)
        wqk = np.stack([r.T for r in qk_rows])           # [8, H, 128]
        wv = np.concatenate(v_rows, axis=0)              # [512, H]
        in_maps.append({
            "xT": xT[b],
            "wqkT": np.ascontiguousarray(wqk).astype(BF16),
            "wvT": np.ascontiguousarray(wv.T).astype(BF16),
            "rope": rope,
            "mtri": mtri,
            "woutT": wout_slices[g],
        })
    return in_maps


def _install_ntff_hook():
    """The image's antenv lacks axon_hooks; shim it so trace=True works."""
    import sys
    import types
    import ctypes
    import contextlib
    if "antenv.axon_hooks" in sys.modules:
        return
    mod = types.ModuleType("antenv.axon_hooks")
    _reg = {"hook": None}
    mod.set_axon_ntff_profile_hook = lambda h: _reg.__setitem__("hook", h)
    mod.get_axon_ntff_profile_hook = lambda: _reg["hook"]
    sys.modules["antenv.axon_hooks"] = mod

    so_path = "/opt/axon/libaxon_pjrt.so"
    try:
        lib = ctypes.CDLL(so_path)
        if not hasattr(lib, "axon_start_nrt_profile"):
            return
        lib.axon_start_nrt_profile.argtypes = [
            ctypes.POINTER(ctypes.c_int64), ctypes.c_size_t]
        lib.axon_start_nrt_profile.restype = ctypes.c_int64
        lib.axon_stop_nrt_profile.argtypes = [ctypes.c_char_p]
        lib.axon_stop_nrt_profile.restype = ctypes.c_int64

        @contextlib.contextmanager
        def _hook(output_dir, device_ids):
            import jax
            jax.devices()
            if device_ids:
                ids = (ctypes.c_int64 * len(device_ids))(*device_ids)
                rc = lib.axon_start_nrt_profile(ids, len(device_ids))
            else:
                rc = lib.axon_start_nrt_profile(None, 0)
            if rc != 0:
                raise RuntimeError(f"axon_start_nrt_profile rc={rc}")
            try:
                yield
            finally:
                n = lib.axon_stop_nrt_profile(str(output_dir).encode())
                print(f"profile: {n} file(s) written to {output_dir}")

        mod.set_axon_ntff_profile_hook(_hook)
    except OSError:
        pass


def _run(in_maps, trace=False):
    if trace:
        _install_ntff_hook()
    from concourse.bass_utils import run_bass_kernel_spmd
    if "nc" not in _cache:
        _cache["nc"] = _build()
    return run_bass_kernel_spmd(_cache["nc"], in_maps,
                                list(range(NCORES)), trace=trace)


def _assemble(r):
    out = np.empty((B, S, H), dtype=np.float32)
    for c in range(NCORES):
        b, g = divmod(c, 4)
        out[b, :, g * 512:(g + 1) * 512] = r.results[c]["out"]
    return out


def kernel(x, attention_mask, frequency_cis, Wqkv, Wout):
    in_maps = _host_prep(x, attention_mask, frequency_cis, Wqkv, Wout)
    r = _run(in_maps)
    return _assemble(r)


def kernel_traced(x, attention_mask, frequency_cis, Wqkv, Wout):
    """Like kernel() but also returns (out, exec_time_ns)."""
    in_maps = _host_prep(x, attention_mask, frequency_cis, Wqkv, Wout)
    r = _run(in_maps, trace=True)
    return _assemble(r), getattr(r, "exec_time_ns", None)
